# revision 6
# baseline (speedup 1.0000x reference)
"""Causal self-attention (B=4, T=2048, C=1024, H=16, D=64) on 8 trn2 NeuronCores.

Sharding: core = 2*b + g  (b = batch index 0..3, g = head-group 0..1).
Each core handles one batch and 8 heads (head-dim columns g*512..g*512+512):
  - QKV projection for its slice (tensor parallel over heads, data parallel on B)
  - flash-style causal attention in S^T layout (keys on partitions)
  - partial output projection  z_partial = y_heads @ W_proj[rows of its heads]
Host unshard: z[b] = z_partial[2b] + z_partial[2b+1] + b_proj.

v2 design (bf16 end-to-end, restructured for engine overlap):
  - all matmul operands bf16 (same PE rate as f32r, half SBUF/DMA)
  - S^T matmuls write bf16 PSUM: one bank per (head-pair, key-chunk) chunk
  - exp split across engines: ScalarE exact exp for diagonal chunks,
    VectorE 1-op Schraudolph exp (int16 bitcast bf16) for off-diagonal
  - diagonal chunks issued FIRST per (ib,pr), their PVs LAST: the causal
    mask multiply (one 128x128 triangular block) never stalls the PE FIFO
  - causal column trim: diag chunk s only computes/exps/PVs cols >= s*128
  - normalization deferred out of the jb loop: reciprocals -> rcp tile,
    one K=65 selector matmul broadcasts per-query recips across partitions,
    single [128,512] multiply per (ib,pr); no DMA in the critical path
  - output projection interleaved per query-block ib (fills PE while
    ScalarE/DVE drain); out DMA spread across the kernel
"""

import math
import os
import sys

import numpy as np

if "/opt/trn_rl_repo" not in sys.path:
    sys.path.insert(0, "/opt/trn_rl_repo")

import concourse.bass as bass
import concourse.bacc as bacc
import concourse.mybir as mybir
import concourse.tile as tile
from concourse.bass_utils import run_bass_kernel_spmd

P = 128
B, C, NH, HD = 4, 1024, 16, 64
T_FULL = 2048
GC = 512          # per-core head-dim columns (8 heads x 64)
TB = 512          # free-dim tile width
NCC = C // P      # 8 contraction chunks for the qkv projection
VB = 193          # vna pair-block width
F32 = mybir.dt.float32
F32R = mybir.dt.float32r
BF16 = mybir.dt.bfloat16
I16 = mybir.dt.int16

# Schraudolph exp constants: exp(s_raw*0.125) ~= bitcast_bf16(int16(s_raw*SA + SB))
SA = 128.0 * 1.4426950408889634 * 0.125
SB = 127.0 * 128.0 - 7.41

_NC_CACHE = {}


def _build(t_len: int, use_mask: bool, loop_n: int = 0, loop_target: str = 'att') -> bass.Bass:
    from contextlib import ExitStack, nullcontext

    ntb = t_len // TB     # query blocks / t blocks
    AOT = mybir.AluOpType

    nc = bacc.Bacc()
    xT = nc.dram_tensor("xT", [C, t_len], BF16, kind="ExternalInput")
    w_qk = nc.dram_tensor("w_qk", [C, 2 * GC], BF16, kind="ExternalInput")
    w_v = nc.dram_tensor("w_v", [C, GC], BF16, kind="ExternalInput")
    w_pr = nc.dram_tensor("w_pr", [GC, C], BF16, kind="ExternalInput")
    consts = nc.dram_tensor("consts", [P, 640], F32, kind="ExternalInput")
    masks = nc.dram_tensor("masks", [P, 2 * P], BF16, kind="ExternalInput")
    vinit = nc.dram_tensor("vinit", [P, 4 * 4 * VB], BF16, kind="ExternalInput")
    bcsel = nc.dram_tensor("bcsel", [P, P], BF16, kind="ExternalInput")
    out = nc.dram_tensor("out", [t_len, C], F32, kind="ExternalOutput")

    with tile.TileContext(nc) as tc, ExitStack() as ctx:
        persist = ctx.enter_context(tc.tile_pool(name="persist", bufs=1))
        qkTs = [persist.tile([P, 2 * GC // P, TB], BF16, tag=f"qkT{tb}", name=f"qkT{tb}")
                for tb in range(ntb)]
        vnas = [persist.tile([P, TB // P, 4 * VB], BF16, tag=f"vna{tb}", name=f"vna{tb}")
                for tb in range(ntb)]
        mk2 = persist.tile([P, 2, P], BF16)       # triangular 128x128 mask, x2 heads
        cst = persist.tile([P, 640], F32)
        bcs = persist.tile([P, P], BF16)
        wpj = persist.tile([P, GC // P, C], BF16)
        rcp = persist.tile([P, ntb, TB], BF16)    # recip denominators, rows 0/64 per pr
        yTs = [persist.tile([P, GC // P, TB], BF16, tag=f"yT{ib}", name=f"yT{ib}")
               for ib in range(ntb)]

        nc.sync.dma_start(cst[:], consts[:])
        nc.sync.dma_start(mk2[:], masks.rearrange("p (s f) -> p s f", s=2))
        nc.sync.dma_start(bcs[:], bcsel[:])
        nc.sync.dma_start(wpj[:], w_pr.rearrange("(o p) n -> p o n", p=P))
        nc.vector.memset(rcp[:], 0.0)
        vinit_r = vinit.rearrange("p (j c) -> p j c", j=4)

        xT_r = xT.rearrange("(o p) t -> p o t", p=P)
        wqk_r = w_qk.rearrange("(o p) m -> p o m", p=P)

        # ---------------- Phase 1: QKV projection ----------------
        with tc.tile_pool(name="ph1w", bufs=1) as ph1w, \
             tc.tile_pool(name="ph1", bufs=2) as ph1, \
             tc.tile_pool(name="wqs", bufs=2) as wqs, \
             tc.tile_pool(name="ps1", bufs=3, space="PSUM") as ps1:
            wv_t = ph1w.tile([P, NCC, GC], BF16, tag="wv")
            nc.sync.dma_start(wv_t[:], w_v.rearrange("(o p) n -> p o n", p=P))

            ph1_loop = (tc.For_i(0, loop_n, 1)
                        if (loop_n and loop_target == "ph1") else nullcontext())
            with ph1_loop:
                for tb in range(ntb):
                  xt = ph1.tile([P, NCC, TB], BF16, tag="xt")
                  nc.sync.dma_start(xt[:], xT_r[:, :, tb * TB:(tb + 1) * TB])
                  nc.sync.dma_start(vnas[tb][:], vinit_r)

                  # V in natural [t, d] layout
                  for tsb in range(TB // P):
                      jc = tb * (TB // P) + tsb
                      ps = ps1.tile([P, TB], F32, tag="ps")
                      for cc in range(NCC):
                          nc.tensor.matmul(
                              ps[:],
                              lhsT=xt[:, cc, tsb * P:(tsb + 1) * P],
                              rhs=wv_t[:, cc, :],
                              start=(cc == 0), stop=(cc == NCC - 1),
                          )
                      vv = vnas[tb][:, tsb, :].rearrange("p (pr c) -> p pr c", c=VB)
                      pr_ps = ps[:].rearrange("p (pr two c) -> p pr two c", two=2, c=64)
                      pr_bv = cst[:, 88:600].rearrange(
                          "p (pr two c) -> p pr two c", two=2, c=64)
                      nc.vector.tensor_tensor(
                          vv[:, :, 0:64], pr_ps[:, :, 0, :], pr_bv[:, :, 0, :], AOT.add
                      )
                      nc.vector.tensor_tensor(
                          vv[:, :, 129:193], pr_ps[:, :, 1, :], pr_bv[:, :, 1, :], AOT.add
                      )
                      if use_mask:
                          nc.vector.tensor_scalar_mul(
                              vnas[tb][:, tsb, :], vnas[tb][:, tsb, :],
                              cst[:, 64 + jc:65 + jc]
                          )

                  # Q^T / K^T rows (transposed layout), two m-blocks per W DMA
                  for mbp in range(GC // P):
                      wq = wqs.tile([P, NCC, 2 * P], BF16, tag="wq")
                      nc.sync.dma_start(
                          wq[:], wqk_r[:, :, mbp * 2 * P:(mbp + 1) * 2 * P])
                      for sub in range(2):
                          mb = 2 * mbp + sub
                          ps = ps1.tile([P, TB], F32, tag="ps")
                          for cc in range(NCC):
                              nc.tensor.matmul(
                                  ps[:],
                                  lhsT=wq[:, cc, sub * P:(sub + 1) * P],
                                  rhs=xt[:, cc, :],
                                  start=(cc == 0), stop=(cc == NCC - 1),
                              )
                          dst = qkTs[tb][:, mb, :]
                          bias = cst[:, 80 + mb:81 + mb]
                          nc.vector.tensor_scalar(dst, ps[:], bias, None, AOT.add)

        # ---------------- Phase 2+3: attention + projection, per ib ----------------
        with tc.tile_pool(name="attd", bufs=4) as attd, \
             tc.tile_pool(name="atto", bufs=3) as atto, \
             tc.tile_pool(name="sps", bufs=3, space="PSUM") as sps, \
             tc.tile_pool(name="pvs", bufs=1, space="PSUM") as pvs, \
             tc.tile_pool(name="opl", bufs=2) as opl:
            loop_ctx = (tc.For_i(0, loop_n, 1)
                        if (loop_n and loop_target == 'att') else nullcontext())
            with loop_ctx:
                for ib in range(ntb):
                    _attention_ib(nc, tc, ib, qkTs, vnas, mk2, bcs, rcp, yTs,
                                  attd, atto, sps, pvs)
                    _proj_ib(nc, tc, ib, yTs, wpj, out, sps, opl)
    nc.finalize()
    return nc


def _attention_ib(nc, tc, ib, qkTs, vnas, mk2, bcs, rcp, yTs,
                  attd, atto, sps, pvs):
    AOT = mybir.AluOpType
    for pr in range(4):             # head pair: heads (2pr, 2pr+1)
        qc, kc = pr, 4 + pr
        # per-head PV accumulators, one PSUM bank each:
        #   pve: [y_e rows 0..63 | denom_e row 64]     (lhsT M=65)
        #   pvo: [denom_o row 0 | zeros | y_o 64..127] (lhsT M=128)
        pve = pvs.tile([P, TB], F32, tag="pve")
        pvo = pvs.tile([P, TB], F32, tag="pvo")

        # --- diagonal chunks first: S + Schraudolph exp + triangular mask ---
        ptbs = []
        for s in range(4):
            c0 = s * P
            sp = sps.tile([P, 2, TB], F32, tag="sp")
            for e in range(2):
                po = 64 * e
                nc.tensor.matmul(
                    sp[:, e, c0:TB],
                    lhsT=qkTs[ib][po:po + 64, kc, s * P:(s + 1) * P],
                    rhs=qkTs[ib][po:po + 64, qc, c0:TB],
                    start=True, stop=True,
                    tile_position=(po, 0),
                )
            pt = attd.tile([P, 2, TB], I16, tag="ptd")
            nc.vector.tensor_scalar(pt[:, :, c0:TB], sp[:, :, c0:TB],
                                    SA, SB, AOT.mult, AOT.add)
            ptb = pt[:].bitcast(BF16)
            nc.vector.tensor_tensor(
                ptb[:, :, c0:c0 + P], ptb[:, :, c0:c0 + P], mk2[:], AOT.mult
            )
            ptbs.append(ptb)

        # --- off-diagonal chunks: S + Schraudolph exp (DVE) + PV ---
        for jb in range(4 * ib):
            tbk, jo = jb // 4, jb % 4
            sp = sps.tile([P, 2, TB], F32, tag="sp")
            for e in range(2):
                po = 64 * e
                nc.tensor.matmul(
                    sp[:, e, :],
                    lhsT=qkTs[tbk][po:po + 64, kc, jo * P:(jo + 1) * P],
                    rhs=qkTs[ib][po:po + 64, qc, :],
                    start=True, stop=True,
                    tile_position=(po, 0),
                )
            pt = atto.tile([P, 2, TB], I16, tag="pto")
            nc.vector.tensor_scalar(pt[:], sp[:], SA, SB, AOT.mult, AOT.add)
            ptb = pt[:].bitcast(BF16)
            nc.tensor.matmul(
                pve[0:65, :],
                lhsT=vnas[tbk][:, jo, pr * VB:pr * VB + 65],
                rhs=ptb[:, 0, :],
                start=(jb == 0), stop=False,
            )
            nc.tensor.matmul(
                pvo[:, :],
                lhsT=vnas[tbk][:, jo, pr * VB + 65:pr * VB + VB],
                rhs=ptb[:, 1, :],
                start=(jb == 0), stop=False,
            )

        # --- diagonal PVs last (exp+mask long done; no PE stall) ---
        for s in range(4):
            c0 = s * P
            ptb = ptbs[s]
            nc.tensor.matmul(
                pve[0:65, c0:TB],
                lhsT=vnas[ib][:, s, pr * VB:pr * VB + 65],
                rhs=ptb[:, 0, c0:TB],
                start=(ib == 0 and s == 0), stop=(s == 3),
            )
            nc.tensor.matmul(
                pvo[:, c0:TB],
                lhsT=vnas[ib][:, s, pr * VB + 65:pr * VB + VB],
                rhs=ptb[:, 1, c0:TB],
                start=(ib == 0 and s == 0), stop=(s == 3),
            )

        # --- drain: reciprocals into rcp (rows 0/64), raw y copies ---
        with nc.allow_low_precision(reason="bf16 operand prep"):
            nc.vector.reciprocal(rcp[64:65, pr, :], pve[64:65, :])
            nc.vector.reciprocal(rcp[0:1, pr, :], pvo[0:1, :])
        nc.scalar.copy(yTs[ib][0:64, pr, :], pve[0:64, :])
        nc.scalar.copy(yTs[ib][64:128, pr, :], pvo[64:128, :])

    # --- deferred normalization: broadcast recips, one multiply per pr ---
    for pr in range(4):
        rb = pvs.tile([P, TB], F32, tag=("pve" if pr % 2 == 0 else "pvo"), name="rb")
        nc.tensor.matmul(
            rb[:],
            lhsT=bcs[0:65, :],
            rhs=rcp[0:65, pr, :],
            start=True, stop=True,
        )
        nc.vector.tensor_tensor(
            yTs[ib][:, pr, :], yTs[ib][:, pr, :], rb[:], AOT.mult
        )


def _proj_ib(nc, tc, ib, yTs, wpj, out, sps, opl):
    for to in range(4):
        tsb = 4 * ib + to
        ot = opl.tile([P, C], F32, tag="ot")
        for nb in range(C // TB):
            ps = sps.tile([P, 2, TB], F32, tag="sp", name="po")
            for dc in range(GC // P):
                nc.tensor.matmul(
                    ps[:, 0, :],
                    lhsT=yTs[ib][:, dc, to * P:(to + 1) * P],
                    rhs=wpj[:, dc, nb * TB:(nb + 1) * TB],
                    start=(dc == 0), stop=(dc == GC // P - 1),
                )
            nc.scalar.copy(ot[:, nb * TB:(nb + 1) * TB], ps[:, 0, :])
        nc.sync.dma_start(out[tsb * P:(tsb + 1) * P, :], ot[:])


def _causal_mask2() -> np.ndarray:
    p = np.arange(P)[:, None]
    f = np.arange(P)[None, :]
    m = (p <= f).astype(np.float32)                  # [128, 128]
    return np.concatenate([m, m], axis=1)            # [128, 256] (both heads)


def _make_in_maps(x, W_attn, b_attn, W_proj, attention_mask, t_len):
    import ml_dtypes
    adt = ml_dtypes.bfloat16
    masks_arr = _causal_mask2().astype(adt)
    bcsel_arr = np.zeros((P, P), np.float32)
    bcsel_arr[64, 0:64] = 1.0      # rb rows 0..63  <- rcp row 64 (recip_e)
    bcsel_arr[0, 64:128] = 1.0     # rb rows 64..127 <- rcp row 0 (recip_o)
    vrow = np.zeros((P, 4 * VB), np.float32)
    for prh in range(4):
        vrow[:, prh * VB + 64] = 1.0
        vrow[:, prh * VB + 65] = 1.0
    vinit = np.ascontiguousarray(np.tile(vrow, (1, 4))).astype(adt)
    in_maps = []
    for core in range(8):
        b, g = core // 2, core % 2
        qcols = slice(g * GC, (g + 1) * GC)
        kcols = slice(C + g * GC, C + (g + 1) * GC)
        vcols = slice(2 * C + g * GC, 2 * C + (g + 1) * GC)

        xTn = np.ascontiguousarray(x[b].T).astype(adt)
        w_qk = np.ascontiguousarray(
            np.concatenate([W_attn[:, qcols], W_attn[:, kcols]], axis=1)
        ).astype(adt)
        w_v = np.ascontiguousarray(W_attn[:, vcols]).astype(adt)
        w_pr = np.ascontiguousarray(W_proj[g * GC:(g + 1) * GC, :]).astype(adt)

        cst = np.zeros((P, 640), np.float32)
        km = attention_mask[b].astype(np.float32).reshape(t_len // P, P).T
        cst[:, 64:64 + t_len // P] = km
        b_qk = np.concatenate([b_attn[qcols], b_attn[kcols]]).astype(np.float32)
        cst[:, 80:88] = b_qk.reshape(8, P).T
        cst[:, 88:600] = np.broadcast_to(b_attn[vcols].astype(np.float32), (P, GC))

        in_maps.append({
            "xT": xTn, "w_qk": w_qk, "w_v": w_v, "w_pr": w_pr,
            "consts": cst, "masks": masks_arr,
            "bcsel": bcsel_arr.astype(adt),
            "vinit": vinit,
        })
    return in_maps


def _run(x, W_attn, b_attn, W_proj, b_proj, attention_mask, trace=False):
    t_len = x.shape[1]
    use_mask = not bool(np.all(attention_mask != 0))
    key = (t_len, use_mask)
    if key not in _NC_CACHE:
        _NC_CACHE[key] = _build(t_len, use_mask)
    nc = _NC_CACHE[key]
    in_maps = _make_in_maps(x, W_attn, b_attn, W_proj, attention_mask, t_len)
    res = run_bass_kernel_spmd(nc, in_maps, list(range(8)), trace=trace)
    outs = [res.results[i]["out"] for i in range(8)]
    bp = b_proj.astype(np.float32)[None, :]
    y = np.stack([outs[2 * b] + outs[2 * b + 1] + bp for b in range(B)]).astype(np.float32)
    return y, res


def kernel(x, W_attn, b_attn, W_proj, b_proj, attention_mask):
    x = np.asarray(x, np.float32)
    W_attn = np.asarray(W_attn, np.float32)
    b_attn = np.asarray(b_attn, np.float32)
    W_proj = np.asarray(W_proj, np.float32)
    b_proj = np.asarray(b_proj, np.float32)
    attention_mask = np.asarray(attention_mask)
    y, _ = _run(x, W_attn, b_attn, W_proj, b_proj, attention_mask)
    return y
